# revision 34
# baseline (speedup 1.0000x reference)
"""Trainium2 Bass kernel for the ACT (Adaptive Computation Time) LSTM model.

Strategy: pure data parallelism across the 8 NeuronCores. The batch (128) is
split into 8 shards of 16; each core runs the full T*M sequential recurrence on
its shard with zero inter-core communication. Final outputs per core are tiny
([2,16]): the fc head output and the per-batch-element ponder sums; the host
concatenates / averages them.

Per-core layout ("transposed"): every per-step tensor keeps the reduction /
feature dimension on SBUF partitions and the local batch (16) in the free
dimension, so element-wise work runs on all 128 vector/scalar lanes:
  hT   [128, 4*16]  column block k holds h[128k:128k+128] for the 16 batch rows
  gates live in 4 PSUM accumulators (i,f,g,o), each [128, 4*16]

The recurrent matmul is weight-stationary (lhsT = W_hh.T tiles [128,128] bf16,
moving = hT chunk [128,16]) so the matmul output lands directly in transposed
layout. The x @ W_ih part (+ bias, via a constant-1 input row) is computed once
per time step straight into the same PSUM banks; subsequent ponder steps
accumulate W_hh @ (h_n - h_{n-1}) on top (telescoping), so the x-part is never
re-added and PSUM is only reset once per time step.
"""

import sys
import numpy as np

if "/opt/trn_rl_repo" not in sys.path:
    sys.path.insert(0, "/opt/trn_rl_repo")

import ml_dtypes  # noqa: E402

import concourse.bass as bass  # noqa: E402
import concourse.bacc as bacc  # noqa: E402
import concourse.tile as tile  # noqa: E402
import concourse.mybir as mybir  # noqa: E402
from concourse.bass_utils import run_bass_kernel_spmd  # noqa: E402
from concourse import library_config  # noqa: E402

F32 = mybir.dt.float32
BF16 = mybir.dt.bfloat16
AF = mybir.ActivationFunctionType
ALU = mybir.AluOpType

B_GLOBAL = 128
T_FULL = 256
I_DIM = 64
H = 512
M_PONDER = 5
NCORES = 8
BL = B_GLOBAL // NCORES  # 16 local batch
G4 = 4 * H  # 2048 gate rows
NKT = H // 128  # 4 h chunks
NMT = G4 // 128  # 16 gate tiles
THRESH = 0.99  # 1 - EPS
UNROLL = 8  # time steps per For_i iteration

# gate tile m -> group (i,f,g,o) = m//4, slot = m%4
# MM processing order per step: g group first, o last (shortens the h tail)
ORDER_M = [8, 9, 10, 11, 0, 1, 2, 3, 4, 5, 6, 7, 12, 13, 14, 15]


def build_nc(T, use_for_i=True):
    """Emit the SPMD program for one core (all cores run the same graph)."""
    nc = bacc.Bacc("TRN2", target_bir_lowering=False, debug=False,
                   num_devices=NCORES)

    xT_d = nc.dram_tensor("xT", [128, T * BL], BF16, kind="ExternalInput")
    whh_d = nc.dram_tensor("whhT", [128, NMT * NKT * 128], BF16,
                           kind="ExternalInput")
    wihx_d = nc.dram_tensor("wihxT", [128, G4], BF16, kind="ExternalInput")
    flag_d = nc.dram_tensor("flagexp", [128, NMT * BL], F32, kind="ExternalInput")
    whalt_d = nc.dram_tensor("whaltT", [128, NKT], BF16, kind="ExternalInput")
    wfc_d = nc.dram_tensor("wfcT", [128, NKT], BF16, kind="ExternalInput")
    consts_d = nc.dram_tensor("consts", [1, 2], F32, kind="ExternalInput")
    out_d = nc.dram_tensor("out", [2, BL], F32, kind="ExternalOutput")

    with tile.TileContext(nc) as tc:
        with (
            tc.tile_pool(name="cpool", bufs=1) as cpool,
            tc.tile_pool(name="spool", bufs=1) as spool,
            tc.tile_pool(name="wpool", bufs=3) as wpool,
            tc.tile_pool(name="rpool", bufs=2) as rpool,
            tc.tile_pool(name="psacc", bufs=1, space="PSUM") as psacc,
            tc.tile_pool(name="pssm", bufs=2, space="PSUM") as pssm,
        ):
            # --- constants ---
            whh_sb = cpool.tile([128, NMT * NKT * 128], BF16)
            wihx_sb = cpool.tile([128, G4], BF16)
            xT_sb = cpool.tile([128, T * BL], BF16)
            flag_sb = cpool.tile([128, NMT * BL], F32)
            whalt_sb = cpool.tile([128, NKT], BF16)
            wfc_sb = cpool.tile([128, NKT], BF16)
            consts_sb = cpool.tile([1, 2], F32)
            row_ones = cpool.tile([1, BL], F32)

            nc.sync.dma_start(whh_sb[:, :], whh_d[:, :])
            nc.sync.dma_start(wihx_sb[:, :], wihx_d[:, :])
            nc.sync.dma_start(xT_sb[:, :], xT_d[:, :])
            nc.sync.dma_start(flag_sb[:, :], flag_d[:, :])
            nc.sync.dma_start(whalt_sb[:, :], whalt_d[:, :])
            nc.sync.dma_start(wfc_sb[:, :], wfc_d[:, :])
            nc.sync.dma_start(consts_sb[:, :], consts_d[:, :])
            nc.vector.memset(row_ones[:, :], 1.0)
            nc.gpsimd.load_library(library_config.proxy)

            # --- persistent state ---
            houtT = spool.tile([128, NKT * BL], BF16)  # h_out(t-1), matmul input
            cc = spool.tile([128, NKT * BL], F32)      # ponder-chain cell state
            hacc = spool.tile([128, NKT * BL], F32)    # sum w_n * h_n
            cacc = spool.tile([128, NKT * BL], F32)    # sum w_n * c_n
            pa = spool.tile([1, BL], F32)              # ponder accumulator

            nc.vector.memset(houtT[:, :], 0.0)
            nc.vector.memset(cacc[:, :], 0.0)
            nc.vector.memset(pa[:, :], 0.0)

            # Prime the activation table with a table that serves BOTH
            # sigmoid and tanh ("sigmoid_and_others"); otherwise the
            # table-load pass greedily picks a tanh-only table for the
            # body's first Tanh and thrashes 2 x 1.3us per time step.
            prime = cpool.tile([1, 1], F32)
            nc.scalar.activation(prime[:, :], row_ones[:, 0:1], AF.Sigmoid)
            nc.scalar.activation(prime[:, :], prime[:, :], AF.Tanh)

            # PSUM accumulators (persist across the whole loop).
            # i,f,o share ONE bank ([0:64|64:128|128:192]) so a single
            # 192-wide SIGMOID serves all three gates; g gets its own bank.
            ps_ifo = psacc.tile([128, 3 * NKT * BL], F32, tag="ps_ifo",
                                name="ps_ifo")
            ps_g = psacc.tile([128, NKT * BL], F32, tag="ps_g", name="ps_g")

            def mm_target(m):
                gi, s = m // 4, m % 4
                if gi == 2:
                    return ps_g[:, s * BL:(s + 1) * BL]
                base = {0: 0, 1: NKT * BL, 3: 2 * NKT * BL}[gi]
                return ps_ifo[:, base + s * BL:base + (s + 1) * BL]

            bhalt_ap = consts_sb[0:1, 0:1]
            bfc_ap = consts_sb[0:1, 1:2]

            def r3(ap):
                return ap.rearrange("p (k b) -> p k b", k=NKT)

            def bookkeeping(n, p, cum_prev, w):
                """ACT halting bookkeeping for step n (tiny [1,16] rows,
                select-free). Returns the new cum_prev."""
                if n == 0:
                    # running=1, prev=0: w = hn ? 1 : p = p + hn*(1-p)
                    hn = rpool.tile([1, BL], F32, tag="hn")
                    d_t = rpool.tile([1, BL], F32, tag="d_t")
                    e_t = rpool.tile([1, BL], F32, tag="e_t")
                    nc.vector.tensor_scalar(hn[:, :], p[:, :], THRESH, None,
                                            ALU.is_ge)
                    nc.vector.tensor_scalar(d_t[:, :], p[:, :], -1.0, 1.0,
                                            ALU.mult, ALU.add)
                    nc.vector.tensor_tensor(e_t[:, :], hn[:, :], d_t[:, :],
                                            ALU.mult)
                    nc.vector.tensor_tensor(w[:, :], p[:, :], e_t[:, :],
                                            ALU.add)
                    nc.vector.tensor_scalar(pa[:, :], pa[:, :], 1.0, None,
                                            ALU.add)
                    nc.vector.tensor_tensor(pa[:, :], pa[:, :], hn[:, :],
                                            ALU.add)
                    return p
                rn = rpool.tile([1, BL], F32, tag="rn")
                rem = rpool.tile([1, BL], F32, tag="rem")
                nc.vector.tensor_scalar(rn[:, :], cum_prev[:, :], THRESH,
                                        None, ALU.is_lt)
                nc.vector.tensor_scalar(rem[:, :], cum_prev[:, :], -1.0,
                                        1.0, ALU.mult, ALU.add)
                nc.vector.tensor_tensor(pa[:, :], pa[:, :], rn[:, :],
                                        ALU.add)
                if n < M_PONDER - 1:
                    cum = rpool.tile([1, BL], F32, tag="cum")
                    ge = rpool.tile([1, BL], F32, tag="ge")
                    hn = rpool.tile([1, BL], F32, tag="hn")
                    nc.vector.tensor_tensor(cum[:, :], cum_prev[:, :],
                                            p[:, :], ALU.add)
                    nc.vector.tensor_scalar(ge[:, :], cum[:, :], THRESH,
                                            None, ALU.is_ge)
                    nc.vector.tensor_tensor(hn[:, :], rn[:, :], ge[:, :],
                                            ALU.mult)
                    # w = hn ? rem : p*rn = pr + hn*(rem - pr)
                    pr = rpool.tile([1, BL], F32, tag="pr")
                    d_t = rpool.tile([1, BL], F32, tag="d_t")
                    e_t = rpool.tile([1, BL], F32, tag="e_t")
                    hr = rpool.tile([1, BL], F32, tag="hr")
                    nc.vector.tensor_tensor(pr[:, :], p[:, :], rn[:, :],
                                            ALU.mult)
                    nc.vector.tensor_tensor(d_t[:, :], rem[:, :],
                                            pr[:, :], ALU.subtract)
                    nc.vector.tensor_tensor(e_t[:, :], hn[:, :],
                                            d_t[:, :], ALU.mult)
                    nc.vector.tensor_tensor(w[:, :], pr[:, :], e_t[:, :],
                                            ALU.add)
                    nc.vector.tensor_tensor(hr[:, :], hn[:, :],
                                            rem[:, :], ALU.mult)
                    nc.vector.tensor_tensor(pa[:, :], pa[:, :],
                                            hr[:, :], ALU.add)
                    return cum
                # forced halt: w = rn*rem, ponder += rn + w
                nc.vector.tensor_tensor(w[:, :], rn[:, :], rem[:, :],
                                        ALU.mult)
                nc.vector.tensor_tensor(pa[:, :], pa[:, :], w[:, :],
                                        ALU.add)
                return None

            def time_step(xcol):
                """Emit one full time step (5 ponder steps)."""
                hT_prev = houtT  # matmul moving operand for n=0
                delta_prev = None
                state = {"cum_prev": None, "pend": None}

                def emit_pending_mm():
                    # p-matmuls for the PREVIOUS ponder step -- emitted in the
                    # middle of the current matmul phase so the (in-order) PE
                    # reaches them well after their hT input is ready.
                    if state["pend"] is None:
                        return
                    pn, phT, pcc = state["pend"]
                    psp = pssm.tile([1, BL], F32, tag="pp")
                    for k in range(NKT):
                        nc.tensor.matmul(
                            psp[:, :],
                            whalt_sb[:, k:k + 1],
                            phT[:, k * BL:(k + 1) * BL],
                            start=(k == 0), stop=(k == NKT - 1),
                        )
                    state["pend"] = (pn, phT, pcc, psp)

                def emit_pending_rest():
                    # sigmoid(p) + bookkeeping + weighted accumulation for the
                    # previous step. Emitted AFTER the current chain's
                    # tg/si/sf so the p-sigmoid doesn't block the chain head
                    # in the (strict-FIFO) ACT queue -- it runs in the gap
                    # while DVE computes the c update.
                    if state["pend"] is None:
                        return
                    pn, phT, pcc, psp = state["pend"]
                    state["pend"] = None
                    p = rpool.tile([1, BL], F32, tag="p")
                    nc.scalar.activation(p[:, :], psp[:, :], AF.Sigmoid,
                                         bias=bhalt_ap)
                    w = rpool.tile([1, BL], F32, tag="w")
                    state["cum_prev"] = bookkeeping(pn, p, state["cum_prev"], w)
                    # weighted accumulation on GpSimd (off critical path)
                    wb = rpool.tile([128, BL], F32, tag="wb")
                    nc.gpsimd.partition_broadcast(wb[:, :], w[:, :])
                    wb_ap = wb[:, :].unsqueeze(1).broadcast_to([128, NKT, BL])
                    if pn == 0:
                        nc.gpsimd.tensor_tensor(r3(hacc[:, :]), r3(phT[:, :]),
                                                wb_ap, ALU.mult)
                        nc.gpsimd.tensor_tensor(r3(cacc[:, :]), r3(pcc[:, :]),
                                                wb_ap, ALU.mult)
                    else:
                        t_hw = wpool.tile([128, NKT * BL], F32, tag="t_hw")
                        t_cw = wpool.tile([128, NKT * BL], F32, tag="t_cw")
                        nc.gpsimd.tensor_tensor(r3(t_hw[:, :]), r3(phT[:, :]),
                                                wb_ap, ALU.mult)
                        nc.gpsimd.tensor_tensor(hacc[:, :], hacc[:, :],
                                                t_hw[:, :], ALU.add)
                        nc.gpsimd.tensor_tensor(r3(t_cw[:, :]), r3(pcc[:, :]),
                                                wb_ap, ALU.mult)
                        nc.gpsimd.tensor_tensor(cacc[:, :], cacc[:, :],
                                                t_cw[:, :], ALU.add)

                for n in range(M_PONDER):
                    # ---- gate matmuls ----
                    if n == 0:
                        # x part (+ bias via ones row): resets PSUM.
                        # start=True zeroes the whole 2KB PSUM bank (one bank
                        # per gate group), so only the first matmul per group
                        # sets it.
                        for m in ORDER_M:
                            nc.tensor.matmul(
                                mm_target(m),
                                wihx_sb[:, m * 128:(m + 1) * 128],
                                xcol,
                                # one start per bank (m=8 opens ps_g,
                                # m=0 opens ps_ifo)
                                start=(m in (8, 0)), stop=False,
                            )
                        moving = houtT
                    else:
                        moving = delta_prev
                    for mi, m in enumerate(ORDER_M):
                        if mi == 4:
                            emit_pending_mm()
                        for k in range(NKT):
                            # n==0 closes each bank's group before the reads;
                            # later steps accumulate with the group check
                            # bypassed (stop is sim-only, no HW effect).
                            nc.tensor.matmul(
                                mm_target(m),
                                whh_sb[:, (m * NKT + k) * 128:(m * NKT + k + 1) * 128],
                                moving[:, k * BL:(k + 1) * BL],
                                start=False,
                                stop=(n == 0 and m in (11, 15) and k == NKT - 1),
                                skip_group_check=(n > 0),
                            )

                    # ---- pointwise ----
                    # The i/f/g->c->h chain is split into half-chunks so the
                    # first half of delta is ready before the current matmul
                    # phase finishes -> the next step's matmuls start with no
                    # PE stall.
                    HB = NKT * BL // 2  # half-chunk width (32 cols)

                    if n == 0:
                        src_ifo = wpool.tile([128, 3 * NKT * BL], F32,
                                             tag="fx_ifo")
                        src_g = wpool.tile([128, NKT * BL], F32, tag="fx_g")
                    else:
                        src_ifo, src_g = ps_ifo, ps_g

                    tg = wpool.tile([128, NKT * BL], F32, tag="tg")
                    sio = wpool.tile([128, 3 * NKT * BL], F32, tag="sio")
                    tcc = wpool.tile([128, NKT * BL], F32, tag="tcc")
                    a_t = wpool.tile([128, NKT * BL], F32, tag="a_t")
                    b_t = wpool.tile([128, NKT * BL], F32, tag="b_t")
                    hT = wpool.tile([128, NKT * BL], BF16, tag="hT")
                    if n < M_PONDER - 1:
                        delta = wpool.tile([128, NKT * BL], BF16, tag="dl",
                                           name="delta")
                    else:
                        delta = None

                    # One 192-wide sigmoid serves i, f and o at once.
                    c_src = cacc if n == 0 else cc
                    if n == 0:
                        nc.vector.tensor_tensor(
                            src_ifo[:, :], ps_ifo[:, :],
                            flag_sb[:, :3 * NKT * BL], ALU.add)
                        nc.vector.tensor_tensor(
                            src_g[:, :], ps_g[:, :],
                            flag_sb[:, 3 * NKT * BL:], ALU.add)
                    nc.scalar.activation(tg[:, :], src_g[:, :], AF.Tanh)
                    nc.scalar.activation(sio[:, :], src_ifo[:, :], AF.Sigmoid)
                    si = sio[:, 0:NKT * BL]
                    sf = sio[:, NKT * BL:2 * NKT * BL]
                    so = sio[:, 2 * NKT * BL:3 * NKT * BL]
                    nc.vector.tensor_tensor(a_t[:, :], si, tg[:, :], ALU.mult)
                    nc.vector.tensor_tensor(b_t[:, :], sf, c_src[:, :],
                                            ALU.mult)
                    emit_pending_rest()
                    for hh in range(2):
                        sl = slice(hh * HB, (hh + 1) * HB)
                        so_sl = sio[:, 2 * NKT * BL + hh * HB:
                                    2 * NKT * BL + (hh + 1) * HB]
                        nc.vector.tensor_tensor(cc[:, sl], a_t[:, sl],
                                                b_t[:, sl], ALU.add)
                        nc.scalar.activation(tcc[:, sl], cc[:, sl], AF.Tanh)
                        nc.vector.tensor_tensor(hT[:, sl], so_sl,
                                                tcc[:, sl], ALU.mult)
                        if delta is not None:
                            nc.vector.tensor_tensor(
                                delta[:, sl], hT[:, sl], hT_prev[:, sl],
                                ALU.subtract)
                    if delta is not None:
                        delta_prev = delta

                    if n < M_PONDER - 1:
                        # p-matmul / bookkeeping / accumulation deferred into
                        # the next matmul phase (see emit_pending)
                        state["pend"] = (n, hT, cc)
                        hT_prev = hT
                    else:
                        # ---- fast time-step boundary ----
                        # w4 = running*(1-prev) needs only cum_3 (p_4 unused),
                        # so it is ready before this matmul phase ends. Fold
                        # the final accumulation straight into houtT/cacc on
                        # DVE: houtT = bf16(hacc_3 + w4*h_4).
                        w4 = rpool.tile([1, BL], F32, tag="w")
                        state["cum_prev"] = bookkeeping(
                            n, None, state["cum_prev"], w4)
                        wb4 = rpool.tile([128, BL], F32, tag="wb")
                        nc.gpsimd.partition_broadcast(wb4[:, :], w4[:, :])
                        wb4_ap = wb4[:, :].unsqueeze(1).broadcast_to(
                            [128, NKT, BL])
                        t_hw = wpool.tile([128, NKT * BL], F32, tag="t_hw")
                        t_cw = wpool.tile([128, NKT * BL], F32, tag="t_cw")
                        nc.vector.tensor_tensor(r3(t_hw[:, :]), r3(hT[:, :]),
                                                wb4_ap, ALU.mult)
                        nc.vector.tensor_tensor(houtT[:, :], hacc[:, :],
                                                t_hw[:, :], ALU.add)
                        nc.vector.tensor_tensor(r3(t_cw[:, :]), r3(cc[:, :]),
                                                wb4_ap, ALU.mult)
                        nc.vector.tensor_tensor(cacc[:, :], cacc[:, :],
                                                t_cw[:, :], ALU.add)

            if use_for_i:
                # unroll UNROLL time steps per dynamic iteration to amortize
                # the ~5us back-edge barrier
                assert T % UNROLL == 0
                with tc.For_i(0, T * BL, UNROLL * BL,
                              hint_engines=(mybir.EngineType.PE,)) as iv:
                    for u in range(UNROLL):
                        time_step(xT_sb[:, bass.ds(iv + u * BL, BL)])
            else:
                for t in range(T):
                    time_step(xT_sb[:, t * BL:(t + 1) * BL])

            # ---- final fc head + outputs ----
            psfc = pssm.tile([1, BL], F32, tag="pp")
            for k in range(NKT):
                nc.tensor.matmul(
                    psfc[:, :],
                    wfc_sb[:, k:k + 1],
                    houtT[:, k * BL:(k + 1) * BL],
                    start=(k == 0), stop=(k == NKT - 1),
                )
            fc_row = rpool.tile([1, BL], F32, tag="fc")
            nc.vector.tensor_scalar(fc_row[:, :], psfc[:, :], bfc_ap, None,
                                    ALU.add)
            nc.sync.dma_start(out_d[0:1, :], fc_row[:, :])
            nc.sync.dma_start(out_d[1:2, :], pa[:, :])

    nc.finalize()
    return nc


def prep_inputs(x, W_ih, W_hh, b, w_halt, b_halt, W_fc, b_fc, T):
    """Host-side shard + layout prep. Returns in_maps for the 8 cores."""
    bf = ml_dtypes.bfloat16
    x = np.asarray(x, np.float32)
    W_ih = np.asarray(W_ih, np.float32)
    W_hh = np.asarray(W_hh, np.float32)
    b = np.asarray(b, np.float32)
    w_halt = np.asarray(w_halt, np.float32)
    b_halt = np.asarray(b_halt, np.float32).reshape(())
    W_fc = np.asarray(W_fc, np.float32)
    b_fc = np.asarray(b_fc, np.float32).reshape(-1)

    # W_hh.T tiles: col block (m*NKT+k) holds W_hh[128m:128m+128, 128k:128k+128].T
    whhT = np.empty((128, NMT * NKT * 128), np.float32)
    for m in range(NMT):
        for k in range(NKT):
            blk = W_hh[m * 128:(m + 1) * 128, k * 128:(k + 1) * 128].T
            whhT[:, (m * NKT + k) * 128:(m * NKT + k + 1) * 128] = blk
    whhT = whhT.astype(bf)

    # W_ih x-part + bias row: [65, 2048]
    wihxT = np.zeros((128, G4), np.float32)
    wihxT[:I_DIM, :] = W_ih[:, :I_DIM].T
    wihxT[I_DIM, :] = b
    wihxT = wihxT.astype(bf)

    # flag column of W_ih expanded to the tiled layout [128, 16*16]:
    # [i(0:64) | f(64:128) | o(128:192) | g(192:256)] to match the PSUM packing
    flagexp = np.empty((128, NMT * BL), np.float32)
    grp_base = {0: 0, 1: NKT * BL, 2: 3 * NKT * BL, 3: 2 * NKT * BL}
    for m in range(NMT):
        off = grp_base[m // 4] + (m % 4) * BL
        flagexp[:, off:off + BL] = \
            W_ih[m * 128:(m + 1) * 128, I_DIM][:, None]

    whaltT = w_halt.reshape(NKT, 128).T.astype(bf).copy()
    wfcT = W_fc.reshape(-1).reshape(NKT, 128).T.astype(bf).copy()
    consts = np.array([[float(b_halt), float(b_fc[0])]], np.float32)

    in_maps = []
    for c in range(NCORES):
        xs = x[c * BL:(c + 1) * BL, :T, :]  # [16, T, 64]
        xT = np.zeros((128, T * BL), np.float32)
        xT[:I_DIM, :] = xs.transpose(2, 1, 0).reshape(I_DIM, T * BL)
        xT[I_DIM, :] = 1.0
        in_maps.append({
            "xT": xT.astype(bf),
            "whhT": whhT,
            "wihxT": wihxT,
            "flagexp": flagexp,
            "whaltT": whaltT,
            "wfcT": wfcT,
            "consts": consts,
        })
    return in_maps


_CACHE = {}


def kernel(x, W_ih, W_hh, b, w_halt, b_halt, W_fc, b_fc):
    T = np.asarray(x).shape[1]
    in_maps = prep_inputs(x, W_ih, W_hh, b, w_halt, b_halt, W_fc, b_fc, T)
    if T not in _CACHE:
        _CACHE[T] = build_nc(T)
    nc = _CACHE[T]
    res = run_bass_kernel_spmd(nc, in_maps, list(range(NCORES)))
    outs = res.results
    out_full = np.empty((B_GLOBAL, 1), np.float32)
    ponder_sum = 0.0
    for c in range(NCORES):
        o = np.asarray(outs[c]["out"], np.float32)
        out_full[c * BL:(c + 1) * BL, 0] = o[0, :]
        ponder_sum += float(o[1, :].sum())
    ponder = np.float32(ponder_sum / (T * B_GLOBAL))
    return out_full, ponder


# revision 38
# speedup vs baseline: 1.1174x; 1.1174x over previous
"""Trainium2 Bass kernel for the ACT (Adaptive Computation Time) LSTM model.

Strategy: pure data parallelism across the 8 NeuronCores. The batch (128) is
split into 8 shards of 16; each core runs the full T*M sequential recurrence on
its shard with zero inter-core communication. Final outputs per core are tiny
([2,16]): the fc head output and the per-batch-element ponder sums; the host
concatenates / averages them.

Per-core layout ("transposed"): every per-step tensor keeps the reduction /
feature dimension on SBUF partitions and the local batch (16) in the free
dimension, so element-wise work runs on all 128 vector/scalar lanes:
  hT   [128, 4*16]  column block k holds h[128k:128k+128] for the 16 batch rows
  gates live in 4 PSUM accumulators (i,f,g,o), each [128, 4*16]

The recurrent matmul is weight-stationary (lhsT = W_hh.T tiles [128,128] bf16,
moving = hT chunk [128,16]) so the matmul output lands directly in transposed
layout. The x @ W_ih part (+ bias, via a constant-1 input row) is computed once
per time step straight into the same PSUM banks; subsequent ponder steps
accumulate W_hh @ (h_n - h_{n-1}) on top (telescoping), so the x-part is never
re-added and PSUM is only reset once per time step.
"""

import sys
import numpy as np

if "/opt/trn_rl_repo" not in sys.path:
    sys.path.insert(0, "/opt/trn_rl_repo")

import ml_dtypes  # noqa: E402

import concourse.bass as bass  # noqa: E402
import concourse.bacc as bacc  # noqa: E402
import concourse.tile as tile  # noqa: E402
import concourse.mybir as mybir  # noqa: E402
from concourse.bass_utils import run_bass_kernel_spmd  # noqa: E402
from concourse import library_config  # noqa: E402

F32 = mybir.dt.float32
BF16 = mybir.dt.bfloat16
AF = mybir.ActivationFunctionType
ALU = mybir.AluOpType

B_GLOBAL = 128
T_FULL = 256
I_DIM = 64
H = 512
M_PONDER = 5
NCORES = 8
BL = B_GLOBAL // NCORES  # 16 local batch
G4 = 4 * H  # 2048 gate rows
NKT = H // 128  # 4 h chunks
NMT = G4 // 128  # 16 gate tiles
THRESH = 0.99  # 1 - EPS
UNROLL = 8  # time steps per For_i iteration

# gate tile m -> group (i,f,g,o) = m//4, slot = m%4
# MM processing order per step: g group first, o last (shortens the h tail)
ORDER_M = [8, 9, 10, 11, 0, 1, 2, 3, 4, 5, 6, 7, 12, 13, 14, 15]


def build_nc(T, use_for_i=True):
    """Emit the SPMD program for one core (all cores run the same graph)."""
    nc = bacc.Bacc("TRN2", target_bir_lowering=False, debug=False,
                   num_devices=NCORES)

    xT_d = nc.dram_tensor("xT", [128, T * BL], BF16, kind="ExternalInput")
    whh_d = nc.dram_tensor("whhT", [128, NMT * NKT * 128], BF16,
                           kind="ExternalInput")
    wihx_d = nc.dram_tensor("wihxT", [128, G4], BF16, kind="ExternalInput")
    flag_d = nc.dram_tensor("flagexp", [128, NMT * BL], F32, kind="ExternalInput")
    whalt_d = nc.dram_tensor("whaltT", [128, NKT], BF16, kind="ExternalInput")
    wfc_d = nc.dram_tensor("wfcT", [128, NKT], BF16, kind="ExternalInput")
    consts_d = nc.dram_tensor("consts", [1, 2], F32, kind="ExternalInput")
    out_d = nc.dram_tensor("out", [2, BL], F32, kind="ExternalOutput")

    with tile.TileContext(nc) as tc:
        with (
            tc.tile_pool(name="cpool", bufs=1) as cpool,
            tc.tile_pool(name="spool", bufs=1) as spool,
            tc.tile_pool(name="wpool", bufs=3) as wpool,
            tc.tile_pool(name="rpool", bufs=2) as rpool,
            tc.tile_pool(name="psacc", bufs=1, space="PSUM") as psacc,
            tc.tile_pool(name="pssm", bufs=2, space="PSUM") as pssm,
        ):
            # --- constants ---
            whh_sb = cpool.tile([128, NMT * NKT * 128], BF16)
            wihx_sb = cpool.tile([128, G4], BF16)
            xT_sb = cpool.tile([128, T * BL], BF16)
            flag_sb = cpool.tile([128, NMT * BL], F32)
            whalt_sb = cpool.tile([128, NKT], BF16)
            wfc_sb = cpool.tile([128, NKT], BF16)
            consts_sb = cpool.tile([1, 2], F32)
            row_ones = cpool.tile([1, BL], F32)

            nc.sync.dma_start(whh_sb[:, :], whh_d[:, :])
            nc.sync.dma_start(wihx_sb[:, :], wihx_d[:, :])
            nc.sync.dma_start(xT_sb[:, :], xT_d[:, :])
            nc.sync.dma_start(flag_sb[:, :], flag_d[:, :])
            nc.sync.dma_start(whalt_sb[:, :], whalt_d[:, :])
            nc.sync.dma_start(wfc_sb[:, :], wfc_d[:, :])
            nc.sync.dma_start(consts_sb[:, :], consts_d[:, :])
            nc.vector.memset(row_ones[:, :], 1.0)
            nc.gpsimd.load_library(library_config.proxy)

            # --- persistent state ---
            houtT = spool.tile([128, NKT * BL], BF16)  # h_out(t-1), matmul input
            cc = spool.tile([128, NKT * BL], F32)      # ponder-chain cell state
            hacc = spool.tile([128, NKT * BL], F32)    # sum w_n * h_n
            cacc = spool.tile([128, NKT * BL], F32)    # sum w_n * c_n
            pa = spool.tile([1, BL], F32)              # ponder accumulator

            nc.vector.memset(houtT[:, :], 0.0)
            nc.vector.memset(cacc[:, :], 0.0)
            nc.vector.memset(pa[:, :], 0.0)

            # Prime the activation table with a table that serves BOTH
            # sigmoid and tanh ("sigmoid_and_others"); otherwise the
            # table-load pass greedily picks a tanh-only table for the
            # body's first Tanh and thrashes 2 x 1.3us per time step.
            prime = cpool.tile([1, 1], F32)
            nc.scalar.activation(prime[:, :], row_ones[:, 0:1], AF.Sigmoid)
            nc.scalar.activation(prime[:, :], prime[:, :], AF.Tanh)

            # PSUM accumulators (persist across the whole loop).
            # i and f share ONE bank ([0:64|64:128]) so a single 128-wide
            # SIGMOID serves both; both complete before the o tiles so the
            # read never overlaps writes of the same bank. o and g keep
            # their own banks.
            ps_if = psacc.tile([128, 2 * NKT * BL], F32, tag="ps_if",
                               name="ps_if")
            ps_o = psacc.tile([128, NKT * BL], F32, tag="ps_o", name="ps_o")
            ps_g = psacc.tile([128, NKT * BL], F32, tag="ps_g", name="ps_g")

            def mm_target(m):
                gi, s = m // 4, m % 4
                if gi == 2:
                    return ps_g[:, s * BL:(s + 1) * BL]
                if gi == 3:
                    return ps_o[:, s * BL:(s + 1) * BL]
                base = gi * NKT * BL
                return ps_if[:, base + s * BL:base + (s + 1) * BL]

            bhalt_ap = consts_sb[0:1, 0:1]
            bfc_ap = consts_sb[0:1, 1:2]

            def r3(ap):
                return ap.rearrange("p (k b) -> p k b", k=NKT)

            def bookkeeping(n, p, cum_prev, w):
                """ACT halting bookkeeping for step n (tiny [1,16] rows,
                select-free). Returns the new cum_prev."""
                if n == 0:
                    # running=1, prev=0: w = hn ? 1 : p = p + hn*(1-p)
                    hn = rpool.tile([1, BL], F32, tag="hn")
                    d_t = rpool.tile([1, BL], F32, tag="d_t")
                    e_t = rpool.tile([1, BL], F32, tag="e_t")
                    nc.vector.tensor_scalar(hn[:, :], p[:, :], THRESH, None,
                                            ALU.is_ge)
                    nc.vector.tensor_scalar(d_t[:, :], p[:, :], -1.0, 1.0,
                                            ALU.mult, ALU.add)
                    nc.vector.tensor_tensor(e_t[:, :], hn[:, :], d_t[:, :],
                                            ALU.mult)
                    nc.vector.tensor_tensor(w[:, :], p[:, :], e_t[:, :],
                                            ALU.add)
                    nc.vector.tensor_scalar(pa[:, :], pa[:, :], 1.0, None,
                                            ALU.add)
                    nc.vector.tensor_tensor(pa[:, :], pa[:, :], hn[:, :],
                                            ALU.add)
                    return p
                rn = rpool.tile([1, BL], F32, tag="rn")
                rem = rpool.tile([1, BL], F32, tag="rem")
                nc.vector.tensor_scalar(rn[:, :], cum_prev[:, :], THRESH,
                                        None, ALU.is_lt)
                nc.vector.tensor_scalar(rem[:, :], cum_prev[:, :], -1.0,
                                        1.0, ALU.mult, ALU.add)
                nc.vector.tensor_tensor(pa[:, :], pa[:, :], rn[:, :],
                                        ALU.add)
                if n < M_PONDER - 1:
                    cum = rpool.tile([1, BL], F32, tag="cum")
                    ge = rpool.tile([1, BL], F32, tag="ge")
                    hn = rpool.tile([1, BL], F32, tag="hn")
                    nc.vector.tensor_tensor(cum[:, :], cum_prev[:, :],
                                            p[:, :], ALU.add)
                    nc.vector.tensor_scalar(ge[:, :], cum[:, :], THRESH,
                                            None, ALU.is_ge)
                    nc.vector.tensor_tensor(hn[:, :], rn[:, :], ge[:, :],
                                            ALU.mult)
                    # w = hn ? rem : p*rn = pr + hn*(rem - pr)
                    pr = rpool.tile([1, BL], F32, tag="pr")
                    d_t = rpool.tile([1, BL], F32, tag="d_t")
                    e_t = rpool.tile([1, BL], F32, tag="e_t")
                    hr = rpool.tile([1, BL], F32, tag="hr")
                    nc.vector.tensor_tensor(pr[:, :], p[:, :], rn[:, :],
                                            ALU.mult)
                    nc.vector.tensor_tensor(d_t[:, :], rem[:, :],
                                            pr[:, :], ALU.subtract)
                    nc.vector.tensor_tensor(e_t[:, :], hn[:, :],
                                            d_t[:, :], ALU.mult)
                    nc.vector.tensor_tensor(w[:, :], pr[:, :], e_t[:, :],
                                            ALU.add)
                    nc.vector.tensor_tensor(hr[:, :], hn[:, :],
                                            rem[:, :], ALU.mult)
                    nc.vector.tensor_tensor(pa[:, :], pa[:, :],
                                            hr[:, :], ALU.add)
                    return cum
                # forced halt: w = rn*rem, ponder += rn + w
                nc.vector.tensor_tensor(w[:, :], rn[:, :], rem[:, :],
                                        ALU.mult)
                nc.vector.tensor_tensor(pa[:, :], pa[:, :], w[:, :],
                                        ALU.add)
                return None

            def time_step(xcol):
                """Emit one full time step (5 ponder steps)."""
                hT_prev = houtT  # matmul moving operand for n=0
                delta_prev = None
                state = {"cum_prev": None, "pend": None}

                def emit_pending_mm():
                    # p-matmuls for the PREVIOUS ponder step -- emitted in the
                    # middle of the current matmul phase so the (in-order) PE
                    # reaches them well after their hT input is ready.
                    if state["pend"] is None:
                        return
                    pn, phT, pcc = state["pend"]
                    psp = pssm.tile([1, BL], F32, tag="pp")
                    for k in range(NKT):
                        nc.tensor.matmul(
                            psp[:, :],
                            whalt_sb[:, k:k + 1],
                            phT[:, k * BL:(k + 1) * BL],
                            start=(k == 0), stop=(k == NKT - 1),
                        )
                    state["pend"] = (pn, phT, pcc, psp)

                def emit_pending_rest():
                    # sigmoid(p) + bookkeeping + weighted accumulation for the
                    # previous step. Emitted AFTER the current chain's
                    # tg/si/sf so the p-sigmoid doesn't block the chain head
                    # in the (strict-FIFO) ACT queue -- it runs in the gap
                    # while DVE computes the c update.
                    if state["pend"] is None:
                        return
                    pn, phT, pcc, psp = state["pend"]
                    state["pend"] = None
                    p = rpool.tile([1, BL], F32, tag="p")
                    nc.scalar.activation(p[:, :], psp[:, :], AF.Sigmoid,
                                         bias=bhalt_ap)
                    w = rpool.tile([1, BL], F32, tag="w")
                    state["cum_prev"] = bookkeeping(pn, p, state["cum_prev"], w)
                    # weighted accumulation on GpSimd (off critical path)
                    wb = rpool.tile([128, BL], F32, tag="wb")
                    nc.gpsimd.partition_broadcast(wb[:, :], w[:, :])
                    wb_ap = wb[:, :].unsqueeze(1).broadcast_to([128, NKT, BL])
                    if pn == 0:
                        nc.gpsimd.tensor_tensor(r3(hacc[:, :]), r3(phT[:, :]),
                                                wb_ap, ALU.mult)
                        nc.gpsimd.tensor_tensor(r3(cacc[:, :]), r3(pcc[:, :]),
                                                wb_ap, ALU.mult)
                    else:
                        t_hw = wpool.tile([128, NKT * BL], F32, tag="t_hw")
                        t_cw = wpool.tile([128, NKT * BL], F32, tag="t_cw")
                        nc.gpsimd.tensor_tensor(r3(t_hw[:, :]), r3(phT[:, :]),
                                                wb_ap, ALU.mult)
                        nc.gpsimd.tensor_tensor(hacc[:, :], hacc[:, :],
                                                t_hw[:, :], ALU.add)
                        nc.gpsimd.tensor_tensor(r3(t_cw[:, :]), r3(pcc[:, :]),
                                                wb_ap, ALU.mult)
                        nc.gpsimd.tensor_tensor(cacc[:, :], cacc[:, :],
                                                t_cw[:, :], ALU.add)

                for n in range(M_PONDER):
                    # ---- gate matmuls ----
                    if n == 0:
                        # x part (+ bias via ones row): resets PSUM.
                        # start=True zeroes the whole 2KB PSUM bank (one bank
                        # per gate group), so only the first matmul per group
                        # sets it.
                        for m in ORDER_M:
                            nc.tensor.matmul(
                                mm_target(m),
                                wihx_sb[:, m * 128:(m + 1) * 128],
                                xcol,
                                # one start per bank (g / if / o)
                                start=(m in (8, 0, 12)), stop=False,
                            )
                        moving = houtT
                    else:
                        moving = delta_prev
                    for mi, m in enumerate(ORDER_M):
                        if mi == 4:
                            emit_pending_mm()
                        for k in range(NKT):
                            # n==0 closes each bank's group before the reads;
                            # later steps accumulate with the group check
                            # bypassed (stop is sim-only, no HW effect).
                            nc.tensor.matmul(
                                mm_target(m),
                                whh_sb[:, (m * NKT + k) * 128:(m * NKT + k + 1) * 128],
                                moving[:, k * BL:(k + 1) * BL],
                                start=False,
                                stop=(n == 0 and m in (11, 7, 15)
                                      and k == NKT - 1),
                                skip_group_check=(n > 0),
                            )

                    # ---- pointwise ----
                    # The i/f/g->c->h chain is split into half-chunks so the
                    # first half of delta is ready before the current matmul
                    # phase finishes -> the next step's matmuls start with no
                    # PE stall.
                    HB = NKT * BL // 2  # half-chunk width (32 cols)

                    if n == 0:
                        src_if = wpool.tile([128, 2 * NKT * BL], F32,
                                            tag="fx_if")
                        src_o = wpool.tile([128, NKT * BL], F32, tag="fx_o")
                        src_g = wpool.tile([128, NKT * BL], F32, tag="fx_g")
                    else:
                        src_if, src_o, src_g = ps_if, ps_o, ps_g

                    tg = wpool.tile([128, NKT * BL], F32, tag="tg")
                    sif = wpool.tile([128, 2 * NKT * BL], F32, tag="sif")
                    so = wpool.tile([128, NKT * BL], F32, tag="so")
                    tcc = wpool.tile([128, NKT * BL], F32, tag="tcc")
                    a_t = wpool.tile([128, NKT * BL], F32, tag="a_t")
                    b_t = wpool.tile([128, NKT * BL], F32, tag="b_t")
                    hT = wpool.tile([128, NKT * BL], BF16, tag="hT")
                    if n < M_PONDER - 1:
                        delta = wpool.tile([128, NKT * BL], BF16, tag="dl",
                                           name="delta")
                    else:
                        delta = None

                    # One 128-wide sigmoid serves i and f at once; o is
                    # activated per half-chunk after its (last) tiles land.
                    c_src = cacc if n == 0 else cc
                    if n == 0:
                        nc.vector.tensor_tensor(
                            src_if[:, :], ps_if[:, :],
                            flag_sb[:, :2 * NKT * BL], ALU.add)
                        nc.vector.tensor_tensor(
                            src_o[:, :], ps_o[:, :],
                            flag_sb[:, 2 * NKT * BL:3 * NKT * BL], ALU.add)
                        nc.vector.tensor_tensor(
                            src_g[:, :], ps_g[:, :],
                            flag_sb[:, 3 * NKT * BL:], ALU.add)
                    nc.scalar.activation(tg[:, :], src_g[:, :], AF.Tanh)
                    nc.scalar.activation(sif[:, :], src_if[:, :], AF.Sigmoid)
                    si = sif[:, 0:NKT * BL]
                    sf = sif[:, NKT * BL:2 * NKT * BL]
                    nc.vector.tensor_tensor(a_t[:, :], si, tg[:, :], ALU.mult)
                    nc.vector.tensor_tensor(b_t[:, :], sf, c_src[:, :],
                                            ALU.mult)
                    emit_pending_rest()
                    for hh in range(2):
                        sl = slice(hh * HB, (hh + 1) * HB)
                        nc.vector.tensor_tensor(cc[:, sl], a_t[:, sl],
                                                b_t[:, sl], ALU.add)
                        nc.scalar.activation(so[:, sl], src_o[:, sl],
                                             AF.Sigmoid)
                        nc.scalar.activation(tcc[:, sl], cc[:, sl], AF.Tanh)
                        nc.vector.tensor_tensor(hT[:, sl], so[:, sl],
                                                tcc[:, sl], ALU.mult)
                        if delta is not None:
                            nc.vector.tensor_tensor(
                                delta[:, sl], hT[:, sl], hT_prev[:, sl],
                                ALU.subtract)
                    if delta is not None:
                        delta_prev = delta

                    if n < M_PONDER - 1:
                        # p-matmul / bookkeeping / accumulation deferred into
                        # the next matmul phase (see emit_pending)
                        state["pend"] = (n, hT, cc)
                        hT_prev = hT
                    else:
                        # ---- fast time-step boundary ----
                        # w4 = running*(1-prev) needs only cum_3 (p_4 unused),
                        # so it is ready before this matmul phase ends. Fold
                        # the final accumulation straight into houtT/cacc on
                        # DVE: houtT = bf16(hacc_3 + w4*h_4).
                        w4 = rpool.tile([1, BL], F32, tag="w")
                        state["cum_prev"] = bookkeeping(
                            n, None, state["cum_prev"], w4)
                        wb4 = rpool.tile([128, BL], F32, tag="wb")
                        nc.gpsimd.partition_broadcast(wb4[:, :], w4[:, :])
                        wb4_ap = wb4[:, :].unsqueeze(1).broadcast_to(
                            [128, NKT, BL])
                        t_hw = wpool.tile([128, NKT * BL], F32, tag="t_hw")
                        t_cw = wpool.tile([128, NKT * BL], F32, tag="t_cw")
                        nc.vector.tensor_tensor(r3(t_hw[:, :]), r3(hT[:, :]),
                                                wb4_ap, ALU.mult)
                        nc.vector.tensor_tensor(houtT[:, :], hacc[:, :],
                                                t_hw[:, :], ALU.add)
                        nc.vector.tensor_tensor(r3(t_cw[:, :]), r3(cc[:, :]),
                                                wb4_ap, ALU.mult)
                        nc.vector.tensor_tensor(cacc[:, :], cacc[:, :],
                                                t_cw[:, :], ALU.add)

            if use_for_i:
                # unroll UNROLL time steps per dynamic iteration to amortize
                # the ~5us back-edge barrier
                assert T % UNROLL == 0
                with tc.For_i(0, T * BL, UNROLL * BL,
                              hint_engines=(mybir.EngineType.PE,)) as iv:
                    for u in range(UNROLL):
                        time_step(xT_sb[:, bass.ds(iv + u * BL, BL)])
            else:
                for t in range(T):
                    time_step(xT_sb[:, t * BL:(t + 1) * BL])

            # ---- final fc head + outputs ----
            psfc = pssm.tile([1, BL], F32, tag="pp")
            for k in range(NKT):
                nc.tensor.matmul(
                    psfc[:, :],
                    wfc_sb[:, k:k + 1],
                    houtT[:, k * BL:(k + 1) * BL],
                    start=(k == 0), stop=(k == NKT - 1),
                )
            fc_row = rpool.tile([1, BL], F32, tag="fc")
            nc.vector.tensor_scalar(fc_row[:, :], psfc[:, :], bfc_ap, None,
                                    ALU.add)
            nc.sync.dma_start(out_d[0:1, :], fc_row[:, :])
            nc.sync.dma_start(out_d[1:2, :], pa[:, :])

    nc.finalize()
    return nc


def prep_inputs(x, W_ih, W_hh, b, w_halt, b_halt, W_fc, b_fc, T):
    """Host-side shard + layout prep. Returns in_maps for the 8 cores."""
    bf = ml_dtypes.bfloat16
    x = np.asarray(x, np.float32)
    W_ih = np.asarray(W_ih, np.float32)
    W_hh = np.asarray(W_hh, np.float32)
    b = np.asarray(b, np.float32)
    w_halt = np.asarray(w_halt, np.float32)
    b_halt = np.asarray(b_halt, np.float32).reshape(())
    W_fc = np.asarray(W_fc, np.float32)
    b_fc = np.asarray(b_fc, np.float32).reshape(-1)

    # W_hh.T tiles: col block (m*NKT+k) holds W_hh[128m:128m+128, 128k:128k+128].T
    whhT = np.empty((128, NMT * NKT * 128), np.float32)
    for m in range(NMT):
        for k in range(NKT):
            blk = W_hh[m * 128:(m + 1) * 128, k * 128:(k + 1) * 128].T
            whhT[:, (m * NKT + k) * 128:(m * NKT + k + 1) * 128] = blk
    whhT = whhT.astype(bf)

    # W_ih x-part + bias row: [65, 2048]
    wihxT = np.zeros((128, G4), np.float32)
    wihxT[:I_DIM, :] = W_ih[:, :I_DIM].T
    wihxT[I_DIM, :] = b
    wihxT = wihxT.astype(bf)

    # flag column of W_ih expanded to the tiled layout [128, 16*16]:
    # [i(0:64) | f(64:128) | o(128:192) | g(192:256)] to match the PSUM packing
    flagexp = np.empty((128, NMT * BL), np.float32)
    grp_base = {0: 0, 1: NKT * BL, 2: 3 * NKT * BL, 3: 2 * NKT * BL}
    for m in range(NMT):
        off = grp_base[m // 4] + (m % 4) * BL
        flagexp[:, off:off + BL] = \
            W_ih[m * 128:(m + 1) * 128, I_DIM][:, None]

    whaltT = w_halt.reshape(NKT, 128).T.astype(bf).copy()
    wfcT = W_fc.reshape(-1).reshape(NKT, 128).T.astype(bf).copy()
    consts = np.array([[float(b_halt), float(b_fc[0])]], np.float32)

    in_maps = []
    for c in range(NCORES):
        xs = x[c * BL:(c + 1) * BL, :T, :]  # [16, T, 64]
        xT = np.zeros((128, T * BL), np.float32)
        xT[:I_DIM, :] = xs.transpose(2, 1, 0).reshape(I_DIM, T * BL)
        xT[I_DIM, :] = 1.0
        in_maps.append({
            "xT": xT.astype(bf),
            "whhT": whhT,
            "wihxT": wihxT,
            "flagexp": flagexp,
            "whaltT": whaltT,
            "wfcT": wfcT,
            "consts": consts,
        })
    return in_maps


_CACHE = {}


def kernel(x, W_ih, W_hh, b, w_halt, b_halt, W_fc, b_fc):
    T = np.asarray(x).shape[1]
    in_maps = prep_inputs(x, W_ih, W_hh, b, w_halt, b_halt, W_fc, b_fc, T)
    if T not in _CACHE:
        _CACHE[T] = build_nc(T)
    nc = _CACHE[T]
    res = run_bass_kernel_spmd(nc, in_maps, list(range(NCORES)))
    outs = res.results
    out_full = np.empty((B_GLOBAL, 1), np.float32)
    ponder_sum = 0.0
    for c in range(NCORES):
        o = np.asarray(outs[c]["out"], np.float32)
        out_full[c * BL:(c + 1) * BL, 0] = o[0, :]
        ponder_sum += float(o[1, :].sum())
    ponder = np.float32(ponder_sum / (T * B_GLOBAL))
    return out_full, ponder


# revision 55
# speedup vs baseline: 1.1585x; 1.0368x over previous
"""Trainium2 Bass kernel for the ACT (Adaptive Computation Time) LSTM model.

Strategy: pure data parallelism across the 8 NeuronCores. The batch (128) is
split into 8 shards of 16; each core runs the full T*M sequential recurrence on
its shard with zero inter-core communication. Final outputs per core are tiny
([2,16]): the fc head output and the per-batch-element ponder sums; the host
concatenates / averages them.

Per-core layout ("transposed"): every per-step tensor keeps the reduction /
feature dimension on SBUF partitions and the local batch (16) in the free
dimension, so element-wise work runs on all 128 vector/scalar lanes:
  hT   [128, 4*16]  column block k holds h[128k:128k+128] for the 16 batch rows
  gates live in 4 PSUM accumulators (i,f,g,o), each [128, 4*16]

The recurrent matmul is weight-stationary (lhsT = W_hh.T tiles [128,128] bf16,
moving = hT chunk [128,16]) so the matmul output lands directly in transposed
layout. The x @ W_ih part (+ bias, via a constant-1 input row) is computed once
per time step straight into the same PSUM banks; subsequent ponder steps
accumulate W_hh @ (h_n - h_{n-1}) on top (telescoping), so the x-part is never
re-added and PSUM is only reset once per time step.
"""

import sys
import numpy as np

if "/opt/trn_rl_repo" not in sys.path:
    sys.path.insert(0, "/opt/trn_rl_repo")

import ml_dtypes  # noqa: E402

import concourse.bass as bass  # noqa: E402
import concourse.bacc as bacc  # noqa: E402
import concourse.tile as tile  # noqa: E402
import concourse.mybir as mybir  # noqa: E402
from concourse.bass_utils import run_bass_kernel_spmd  # noqa: E402
from concourse import library_config  # noqa: E402
from concourse.tile_rust import add_dep_helper  # noqa: E402

F32 = mybir.dt.float32
BF16 = mybir.dt.bfloat16
AF = mybir.ActivationFunctionType
ALU = mybir.AluOpType

B_GLOBAL = 128
T_FULL = 256
I_DIM = 64
H = 512
M_PONDER = 5
NCORES = 8
BL = B_GLOBAL // NCORES  # 16 local batch
G4 = 4 * H  # 2048 gate rows
NKT = H // 128  # 4 h chunks
NMT = G4 // 128  # 16 gate tiles
THRESH = 0.99  # 1 - EPS
UNROLL = 8  # time steps per For_i iteration

# gate tile m -> group (i,f,g,o) = m//4, slot = m%4
# MM processing order per step: g group first, o last (shortens the h tail)
ORDER_M = [8, 9, 10, 11, 0, 1, 2, 3, 4, 5, 6, 7, 12, 13, 14, 15]


def build_nc(T, use_for_i=True):
    """Emit the SPMD program for one core (all cores run the same graph)."""
    nc = bacc.Bacc("TRN2", target_bir_lowering=False, debug=False,
                   num_devices=NCORES)

    xT_d = nc.dram_tensor("xT", [128, T * BL], BF16, kind="ExternalInput")
    whh_d = nc.dram_tensor("whhT", [128, NMT * NKT * 128], BF16,
                           kind="ExternalInput")
    wihx_d = nc.dram_tensor("wihxT", [128, G4], BF16, kind="ExternalInput")
    flag_d = nc.dram_tensor("flagexp", [128, NMT * BL], F32, kind="ExternalInput")
    whalt_d = nc.dram_tensor("whaltT", [128, NKT], BF16, kind="ExternalInput")
    wfc_d = nc.dram_tensor("wfcT", [128, NKT], BF16, kind="ExternalInput")
    consts_d = nc.dram_tensor("consts", [1, 2], F32, kind="ExternalInput")
    out_d = nc.dram_tensor("out", [2, BL], F32, kind="ExternalOutput")

    with tile.TileContext(nc) as tc:
        with (
            tc.tile_pool(name="cpool", bufs=1) as cpool,
            tc.tile_pool(name="spool", bufs=1) as spool,
            tc.tile_pool(name="wpool", bufs=4) as wpool,
            tc.tile_pool(name="rpool", bufs=2) as rpool,
            tc.tile_pool(name="psacc", bufs=1, space="PSUM") as psacc,
            tc.tile_pool(name="pssm", bufs=3, space="PSUM") as pssm,
        ):
            # --- constants ---
            whh_sb = cpool.tile([128, NMT * NKT * 128], BF16)
            wihx_sb = cpool.tile([128, G4], BF16)
            xT_sb = cpool.tile([128, T * BL], BF16)
            flag_sb = cpool.tile([128, NMT * BL], F32)
            whalt_sb = cpool.tile([128, NKT], BF16)
            wfc_sb = cpool.tile([128, NKT], BF16)
            consts_sb = cpool.tile([1, 2], F32)
            row_ones = cpool.tile([1, BL], F32)

            nc.sync.dma_start(whh_sb[:, :], whh_d[:, :])
            nc.sync.dma_start(wihx_sb[:, :], wihx_d[:, :])
            nc.sync.dma_start(xT_sb[:, :], xT_d[:, :])
            nc.sync.dma_start(flag_sb[:, :], flag_d[:, :])
            nc.sync.dma_start(whalt_sb[:, :], whalt_d[:, :])
            nc.sync.dma_start(wfc_sb[:, :], wfc_d[:, :])
            nc.sync.dma_start(consts_sb[:, :], consts_d[:, :])
            nc.vector.memset(row_ones[:, :], 1.0)
            nc.gpsimd.load_library(library_config.proxy)

            # --- persistent state ---
            houtT = spool.tile([128, NKT * BL], BF16)  # h_out(t-1), matmul input
            cc = spool.tile([128, NKT * BL], F32)      # ponder-chain cell state
            hacc = spool.tile([128, NKT * BL], F32)    # sum w_n * h_n
            cacc = spool.tile([128, NKT * BL], F32)    # sum w_n * c_n
            pa = spool.tile([1, BL], F32)              # ponder accumulator

            nc.vector.memset(houtT[:, :], 0.0)
            nc.vector.memset(cacc[:, :], 0.0)
            nc.vector.memset(pa[:, :], 0.0)

            # Prime the activation table with a table that serves BOTH
            # sigmoid and tanh ("sigmoid_and_others"); otherwise the
            # table-load pass greedily picks a tanh-only table for the
            # body's first Tanh and thrashes 2 x 1.3us per time step.
            prime = cpool.tile([1, 1], F32)
            nc.scalar.activation(prime[:, :], row_ones[:, 0:1], AF.Sigmoid)
            nc.scalar.activation(prime[:, :], prime[:, :], AF.Tanh)

            # PSUM accumulators (persist across the whole loop); one bank
            # per gate group. (Merging groups into shared banks to batch
            # the sigmoids was measured SLOWER -- it delays the c-chain
            # past the matmul overlap the separate ops enjoy.)
            ps = {
                g: psacc.tile([128, NKT * BL], F32, tag=f"ps_{g}",
                              name=f"ps_{g}")
                for g in "ifgo"
            }

            def mm_target(m):
                gi, s = m // 4, m % 4
                return ps[" ifgo"[gi + 1]][:, s * BL:(s + 1) * BL]

            bhalt_ap = consts_sb[0:1, 0:1]
            bfc_ap = consts_sb[0:1, 1:2]

            def r3(ap):
                return ap.rearrange("p (k b) -> p k b", k=NKT)

            def bookkeeping(n, p, cum_prev, w):
                """ACT halting bookkeeping for step n (tiny [1,16] rows,
                select-free). Returns the new cum_prev."""
                if n == 0:
                    # running=1, prev=0: w = hn ? 1 : p = p + hn*(1-p)
                    hn = rpool.tile([1, BL], F32, tag="hn")
                    d_t = rpool.tile([1, BL], F32, tag="d_t")
                    e_t = rpool.tile([1, BL], F32, tag="e_t")
                    nc.vector.tensor_scalar(hn[:, :], p[:, :], THRESH, None,
                                            ALU.is_ge)
                    nc.vector.tensor_scalar(d_t[:, :], p[:, :], -1.0, 1.0,
                                            ALU.mult, ALU.add)
                    nc.vector.tensor_tensor(e_t[:, :], hn[:, :], d_t[:, :],
                                            ALU.mult)
                    nc.vector.tensor_tensor(w[:, :], p[:, :], e_t[:, :],
                                            ALU.add)
                    nc.vector.tensor_scalar(pa[:, :], pa[:, :], 1.0, None,
                                            ALU.add)
                    nc.vector.tensor_tensor(pa[:, :], pa[:, :], hn[:, :],
                                            ALU.add)
                    return p
                rn = rpool.tile([1, BL], F32, tag="rn")
                rem = rpool.tile([1, BL], F32, tag="rem")
                nc.vector.tensor_scalar(rn[:, :], cum_prev[:, :], THRESH,
                                        None, ALU.is_lt)
                nc.vector.tensor_scalar(rem[:, :], cum_prev[:, :], -1.0,
                                        1.0, ALU.mult, ALU.add)
                nc.vector.tensor_tensor(pa[:, :], pa[:, :], rn[:, :],
                                        ALU.add)
                if n < M_PONDER - 1:
                    cum = rpool.tile([1, BL], F32, tag="cum")
                    ge = rpool.tile([1, BL], F32, tag="ge")
                    hn = rpool.tile([1, BL], F32, tag="hn")
                    nc.vector.tensor_tensor(cum[:, :], cum_prev[:, :],
                                            p[:, :], ALU.add)
                    nc.vector.tensor_scalar(ge[:, :], cum[:, :], THRESH,
                                            None, ALU.is_ge)
                    nc.vector.tensor_tensor(hn[:, :], rn[:, :], ge[:, :],
                                            ALU.mult)
                    # w = hn ? rem : p*rn = pr + hn*(rem - pr)
                    pr = rpool.tile([1, BL], F32, tag="pr")
                    d_t = rpool.tile([1, BL], F32, tag="d_t")
                    e_t = rpool.tile([1, BL], F32, tag="e_t")
                    hr = rpool.tile([1, BL], F32, tag="hr")
                    nc.vector.tensor_tensor(pr[:, :], p[:, :], rn[:, :],
                                            ALU.mult)
                    nc.vector.tensor_tensor(d_t[:, :], rem[:, :],
                                            pr[:, :], ALU.subtract)
                    nc.vector.tensor_tensor(e_t[:, :], hn[:, :],
                                            d_t[:, :], ALU.mult)
                    nc.vector.tensor_tensor(w[:, :], pr[:, :], e_t[:, :],
                                            ALU.add)
                    nc.vector.tensor_tensor(hr[:, :], hn[:, :],
                                            rem[:, :], ALU.mult)
                    nc.vector.tensor_tensor(pa[:, :], pa[:, :],
                                            hr[:, :], ALU.add)
                    return cum
                # forced halt: w = rn*rem, ponder += rn + w
                nc.vector.tensor_tensor(w[:, :], rn[:, :], rem[:, :],
                                        ALU.mult)
                nc.vector.tensor_tensor(pa[:, :], pa[:, :], w[:, :],
                                        ALU.add)
                return None

            def time_step(xcol):
                """Emit one full time step (5 ponder steps)."""
                hT_prev = houtT  # matmul moving operand for n=0
                delta_prev = None
                state = {"cum_prev": None, "pend": None}

                def emit_pending_mm():
                    # p-matmuls for the PREVIOUS ponder step -- emitted in the
                    # middle of the current matmul phase so the (in-order) PE
                    # reaches them well after their hT input is ready.
                    if state["pend"] is None:
                        return
                    pn, phT, pcc = state["pend"]
                    psp = pssm.tile([1, BL], F32, tag="pp")
                    for k in range(NKT):
                        nc.tensor.matmul(
                            psp[:, :],
                            whalt_sb[:, k:k + 1],
                            phT[:, k * BL:(k + 1) * BL],
                            start=(k == 0), stop=(k == NKT - 1),
                        )
                    state["pend"] = (pn, phT, pcc, psp)

                def emit_pending_rest():
                    # sigmoid(p) + bookkeeping + weighted accumulation for the
                    # previous step. Emitted AFTER the current chain's
                    # tg/si/sf so the p-sigmoid doesn't block the chain head
                    # in the (strict-FIFO) ACT queue -- it runs in the gap
                    # while DVE computes the c update.
                    if state["pend"] is None:
                        return
                    pn, phT, pcc, psp = state["pend"]
                    state["pend"] = None
                    p = rpool.tile([1, BL], F32, tag="p")
                    nc.scalar.activation(p[:, :], psp[:, :], AF.Sigmoid,
                                         bias=bhalt_ap)
                    w = rpool.tile([1, BL], F32, tag="w")
                    state["cum_prev"] = bookkeeping(pn, p, state["cum_prev"], w)
                    # weighted accumulation on GpSimd (off critical path)
                    wb = rpool.tile([128, BL], F32, tag="wb")
                    nc.gpsimd.partition_broadcast(wb[:, :], w[:, :])
                    wb_ap = wb[:, :].unsqueeze(1).broadcast_to([128, NKT, BL])
                    if pn == 0:
                        nc.gpsimd.tensor_tensor(r3(hacc[:, :]), r3(phT[:, :]),
                                                wb_ap, ALU.mult)
                        nc.gpsimd.tensor_tensor(r3(cacc[:, :]), r3(pcc[:, :]),
                                                wb_ap, ALU.mult)
                    else:
                        t_hw = wpool.tile([128, NKT * BL], F32, tag="t_hw")
                        t_cw = wpool.tile([128, NKT * BL], F32, tag="t_cw")
                        nc.gpsimd.tensor_tensor(r3(t_hw[:, :]), r3(phT[:, :]),
                                                wb_ap, ALU.mult)
                        nc.gpsimd.tensor_tensor(hacc[:, :], hacc[:, :],
                                                t_hw[:, :], ALU.add)
                        nc.gpsimd.tensor_tensor(r3(t_cw[:, :]), r3(pcc[:, :]),
                                                wb_ap, ALU.mult)
                        nc.gpsimd.tensor_tensor(cacc[:, :], cacc[:, :],
                                                t_cw[:, :], ALU.add)

                for n in range(M_PONDER):
                    # ---- gate matmuls ----
                    if n == 0:
                        # x part (+ bias via ones row): resets PSUM.
                        # start=True zeroes the whole 2KB PSUM bank (one bank
                        # per gate group), so only the first matmul per group
                        # sets it.
                        for m in ORDER_M:
                            nc.tensor.matmul(
                                mm_target(m),
                                wihx_sb[:, m * 128:(m + 1) * 128],
                                xcol,
                                # one start per bank (one bank per group)
                                start=(m % 4 == 0), stop=False,
                            )
                        moving = houtT
                    else:
                        moving = delta_prev
                    for mi, m in enumerate(ORDER_M):
                        if mi == 4:
                            emit_pending_mm()
                        for k in range(NKT):
                            # n==0 closes each bank's group before the reads;
                            # later steps accumulate with the group check
                            # bypassed (stop is sim-only, no HW effect).
                            nc.tensor.matmul(
                                mm_target(m),
                                whh_sb[:, (m * NKT + k) * 128:(m * NKT + k + 1) * 128],
                                moving[:, k * BL:(k + 1) * BL],
                                start=False,
                                stop=(n == 0 and m % 4 == 3
                                      and k == NKT - 1),
                                skip_group_check=(n > 0),
                            )

                    # ---- pointwise ----
                    # The i/f/g->c->h chain is split into half-chunks so the
                    # first half of delta is ready before the current matmul
                    # phase finishes -> the next step's matmuls start with no
                    # PE stall.
                    HB = NKT * BL // 2  # half-chunk width (32 cols)

                    if n == 0:
                        srcs = {}
                        for gi, g in enumerate("ifgo"):
                            tmp = wpool.tile([128, NKT * BL], F32,
                                             tag=f"fx_{g}", name=f"fx_{g}")
                            srcs[g] = tmp
                    else:
                        srcs = ps

                    tg = wpool.tile([128, NKT * BL], F32, tag="tg")
                    si = wpool.tile([128, NKT * BL], F32, tag="si")
                    sf = wpool.tile([128, NKT * BL], F32, tag="sf")
                    so = wpool.tile([128, NKT * BL], F32, tag="so")
                    tcc = wpool.tile([128, NKT * BL], F32, tag="tcc")
                    a_t = wpool.tile([128, NKT * BL], F32, tag="a_t")
                    b_t = wpool.tile([128, NKT * BL], F32, tag="b_t")
                    hT = wpool.tile([128, NKT * BL], BF16, tag="hT")
                    if n < M_PONDER - 1:
                        delta = wpool.tile([128, NKT * BL], BF16, tag="dl",
                                           name="delta")
                    else:
                        delta = None

                    # full-width chain (per-op fixed costs dominate, so fewer
                    # bigger ops beat fine chunking); sig_o runs per
                    # half-chunk so h/delta pipeline into the next matmuls.
                    c_src = cacc if n == 0 else cc
                    if n == 0:
                        base = {"i": 0, "f": 1, "o": 2, "g": 3}
                        for g in "ifgo":
                            o0 = base[g] * NKT * BL
                            nc.vector.tensor_tensor(
                                srcs[g][:, :], ps[g][:, :],
                                flag_sb[:, o0:o0 + NKT * BL], ALU.add)
                    nc.scalar.activation(tg[:, :], srcs["g"][:, :], AF.Tanh)
                    nc.scalar.activation(si[:, :], srcs["i"][:, :], AF.Sigmoid)
                    nc.vector.tensor_tensor(a_t[:, :], si[:, :], tg[:, :],
                                            ALU.mult)
                    nc.scalar.activation(sf[:, :], srcs["f"][:, :], AF.Sigmoid)
                    nc.vector.tensor_tensor(b_t[:, :], sf[:, :], c_src[:, :],
                                            ALU.mult)
                    emit_pending_rest()
                    for hh in range(2):
                        sl = slice(hh * HB, (hh + 1) * HB)
                        nc.vector.tensor_tensor(cc[:, sl], a_t[:, sl],
                                                b_t[:, sl], ALU.add)
                        nc.scalar.activation(so[:, sl], srcs["o"][:, sl],
                                             AF.Sigmoid)
                        nc.scalar.activation(tcc[:, sl], cc[:, sl], AF.Tanh)
                        nc.vector.tensor_tensor(hT[:, sl], so[:, sl],
                                                tcc[:, sl], ALU.mult)
                        if delta is not None:
                            nc.vector.tensor_tensor(
                                delta[:, sl], hT[:, sl], hT_prev[:, sl],
                                ALU.subtract)
                    if delta is not None:
                        delta_prev = delta

                    if n < M_PONDER - 1:
                        # p-matmul / bookkeeping / accumulation deferred into
                        # the next matmul phase (see emit_pending)
                        state["pend"] = (n, hT, cc)
                        hT_prev = hT
                    else:
                        # ---- fast time-step boundary ----
                        # w4 = running*(1-prev) needs only cum_3 (p_4 unused),
                        # so it is ready before this matmul phase ends. Fold
                        # the final accumulation straight into houtT/cacc on
                        # DVE: houtT = bf16(hacc_3 + w4*h_4).
                        w4 = rpool.tile([1, BL], F32, tag="w")
                        state["cum_prev"] = bookkeeping(
                            n, None, state["cum_prev"], w4)
                        wb4 = rpool.tile([128, BL], F32, tag="wb")
                        nc.gpsimd.partition_broadcast(wb4[:, :], w4[:, :])
                        wb4_ap = wb4[:, :].unsqueeze(1).broadcast_to(
                            [128, NKT, BL])
                        t_hw = wpool.tile([128, NKT * BL], F32, tag="t_hw")
                        t_cw = wpool.tile([128, NKT * BL], F32, tag="t_cw")
                        nc.vector.tensor_tensor(r3(t_hw[:, :]), r3(hT[:, :]),
                                                wb4_ap, ALU.mult)
                        nc.vector.tensor_tensor(houtT[:, :], hacc[:, :],
                                                t_hw[:, :], ALU.add)
                        nc.vector.tensor_tensor(r3(t_cw[:, :]), r3(cc[:, :]),
                                                wb4_ap, ALU.mult)
                        nc.vector.tensor_tensor(cacc[:, :], cacc[:, :],
                                                t_cw[:, :], ALU.add)

            if use_for_i:
                # unroll UNROLL time steps per dynamic iteration to amortize
                # the ~5us back-edge barrier
                assert T % UNROLL == 0
                with tc.For_i(0, T * BL, UNROLL * BL,
                              hint_engines=(mybir.EngineType.PE,)) as iv:
                    for u in range(UNROLL):
                        time_step(xT_sb[:, bass.ds(iv + u * BL, BL)])
            else:
                for t in range(T):
                    time_step(xT_sb[:, t * BL:(t + 1) * BL])

            # ---- final fc head + outputs ----
            psfc = pssm.tile([1, BL], F32, tag="pp")
            for k in range(NKT):
                nc.tensor.matmul(
                    psfc[:, :],
                    wfc_sb[:, k:k + 1],
                    houtT[:, k * BL:(k + 1) * BL],
                    start=(k == 0), stop=(k == NKT - 1),
                )
            fc_row = rpool.tile([1, BL], F32, tag="fc")
            nc.vector.tensor_scalar(fc_row[:, :], psfc[:, :], bfc_ap, None,
                                    ALU.add)
            nc.sync.dma_start(out_d[0:1, :], fc_row[:, :])
            nc.sync.dma_start(out_d[1:2, :], pa[:, :])

    nc.finalize()
    return nc


def prep_inputs(x, W_ih, W_hh, b, w_halt, b_halt, W_fc, b_fc, T):
    """Host-side shard + layout prep. Returns in_maps for the 8 cores."""
    bf = ml_dtypes.bfloat16
    x = np.asarray(x, np.float32)
    W_ih = np.asarray(W_ih, np.float32)
    W_hh = np.asarray(W_hh, np.float32)
    b = np.asarray(b, np.float32)
    w_halt = np.asarray(w_halt, np.float32)
    b_halt = np.asarray(b_halt, np.float32).reshape(())
    W_fc = np.asarray(W_fc, np.float32)
    b_fc = np.asarray(b_fc, np.float32).reshape(-1)

    # W_hh.T tiles: col block (m*NKT+k) holds W_hh[128m:128m+128, 128k:128k+128].T
    whhT = np.empty((128, NMT * NKT * 128), np.float32)
    for m in range(NMT):
        for k in range(NKT):
            blk = W_hh[m * 128:(m + 1) * 128, k * 128:(k + 1) * 128].T
            whhT[:, (m * NKT + k) * 128:(m * NKT + k + 1) * 128] = blk
    whhT = whhT.astype(bf)

    # W_ih x-part + bias row: [65, 2048]
    wihxT = np.zeros((128, G4), np.float32)
    wihxT[:I_DIM, :] = W_ih[:, :I_DIM].T
    wihxT[I_DIM, :] = b
    wihxT = wihxT.astype(bf)

    # flag column of W_ih expanded to the tiled layout [128, 16*16]:
    # [i(0:64) | f(64:128) | o(128:192) | g(192:256)] to match the PSUM packing
    flagexp = np.empty((128, NMT * BL), np.float32)
    grp_base = {0: 0, 1: NKT * BL, 2: 3 * NKT * BL, 3: 2 * NKT * BL}
    for m in range(NMT):
        off = grp_base[m // 4] + (m % 4) * BL
        flagexp[:, off:off + BL] = \
            W_ih[m * 128:(m + 1) * 128, I_DIM][:, None]

    whaltT = w_halt.reshape(NKT, 128).T.astype(bf).copy()
    wfcT = W_fc.reshape(-1).reshape(NKT, 128).T.astype(bf).copy()
    consts = np.array([[float(b_halt), float(b_fc[0])]], np.float32)

    in_maps = []
    for c in range(NCORES):
        xs = x[c * BL:(c + 1) * BL, :T, :]  # [16, T, 64]
        xT = np.zeros((128, T * BL), np.float32)
        xT[:I_DIM, :] = xs.transpose(2, 1, 0).reshape(I_DIM, T * BL)
        xT[I_DIM, :] = 1.0
        in_maps.append({
            "xT": xT.astype(bf),
            "whhT": whhT,
            "wihxT": wihxT,
            "flagexp": flagexp,
            "whaltT": whaltT,
            "wfcT": wfcT,
            "consts": consts,
        })
    return in_maps


_CACHE = {}


def kernel(x, W_ih, W_hh, b, w_halt, b_halt, W_fc, b_fc):
    T = np.asarray(x).shape[1]
    in_maps = prep_inputs(x, W_ih, W_hh, b, w_halt, b_halt, W_fc, b_fc, T)
    if T not in _CACHE:
        _CACHE[T] = build_nc(T)
    nc = _CACHE[T]
    res = run_bass_kernel_spmd(nc, in_maps, list(range(NCORES)))
    outs = res.results
    out_full = np.empty((B_GLOBAL, 1), np.float32)
    ponder_sum = 0.0
    for c in range(NCORES):
        o = np.asarray(outs[c]["out"], np.float32)
        out_full[c * BL:(c + 1) * BL, 0] = o[0, :]
        ponder_sum += float(o[1, :].sum())
    ponder = np.float32(ponder_sum / (T * B_GLOBAL))
    return out_full, ponder
